# revision 26
# baseline (speedup 1.0000x reference)
"""Causal self-attention with anchor-relative rope (ferope), 8-core TRN2 Bass kernel.

Full-scale problem: B=2, T=2048, C=2048, H=16, D=128, M=32.

Sharding (tensor-parallel heads + data-parallel batch), collective-free:
  - 8 cores = 2 batch groups x 4 cores. Core (b, g) handles batch b, heads 4g..4g+3.
  - All matrix inputs are pre-converted to bf16 and pre-permuted on the host so
    each contraction block [128, .] DMAs contiguously into SBUF (no staging).
  - qkv projection: per-core column shard of w_attn; q/k produced in [d, t]
    layout, v in [t, d]; rope applied per 512-panel right after projection.
  - attention runs query-panel-outer / head-inner with transposed scores
    s_T[ki, qi]; diagonal blocks narrowed to the exact causal triangle;
    softmax denominator accumulated in bf16 on the vector engine + one
    ones-matmul per panel; finalization deferred one unit to avoid stalls.
  - output projection needs no AllGather: each core computes the full-width
    partial out^T = wo_own^T @ y_own (same flops as a sharded projection,
    contraction over its own 512 channels) and the host sums the 4 partials
    per batch group while unsharding.
"""

import math

import numpy as np

import concourse.bass as bass
import concourse.mybir as mybir
import concourse.tile as tile
from concourse import bacc
from concourse.bass_utils import run_bass_kernel_spmd

F32 = mybir.dt.float32
BF16 = mybir.dt.bfloat16

# full-scale dims (hardcoded per harness contract)
B, T, C, H, DH, M = 2, 2048, 2048, 16, 128, 32
N_CORES = 8
GROUPS = 2                     # batch groups
CPG = N_CORES // GROUPS        # cores per group = 4
HPC = H // CPG                 # heads per core = 4
C_LOC = HPC * DH               # 512: per-core head channels
PANEL = 512                    # qi panel width (one psum bank)
KB = 128                       # ki block (partition dim)


def build_program():
    n_cb = C // KB              # 16 contraction blocks for qkv
    n_oc = C // KB              # 16 output-column blocks for proj
    n_panels = T // PANEL       # 4
    kb_per_panel = PANEL // KB  # 4
    inv_sqrt_d = 1.0 / math.sqrt(DH)

    nc = bacc.Bacc("TRN2", target_bir_lowering=False, debug=False,
                   num_devices=N_CORES)

    # pre-permuted bf16 inputs, contiguous per partition row so each tensor
    # loads with one big-descriptor DMA. x is quarter-major: [qt, p, kb, 512]
    xT_d = nc.dram_tensor("xT", [n_panels, KB, n_cb, PANEL], BF16,
                          kind="ExternalInput").ap()
    wqk_d = nc.dram_tensor("wqk", [KB, n_cb, 2 * C_LOC], BF16,
                           kind="ExternalInput").ap()
    wv_d = nc.dram_tensor("wv", [KB, n_cb, C_LOC], BF16,
                          kind="ExternalInput").ap()
    # proj weight rows for this core's channels: [p, own-cblk, out-cols]
    wo_d = nc.dram_tensor("wo", [KB, HPC, C], BF16, kind="ExternalInput").ap()
    freqs_d = nc.dram_tensor("freqs", [M], F32, kind="ExternalInput").ap()
    delta_d = nc.dram_tensor("delta", [T], F32, kind="ExternalInput").ap()
    # full-width transposed partial projection; host sums over the group
    partT_d = nc.dram_tensor("partT", [C, T], BF16, kind="ExternalOutput").ap()

    with tile.TileContext(nc) as tc:
        with (
            tc.tile_pool(name="const", bufs=1) as const,
            tc.tile_pool(name="qkv", bufs=1) as qkv,
            tc.tile_pool(name="work", bufs=1) as work,
        ):
            # persistent attention operands
            q_sb = [qkv.tile([DH, T], BF16, name=f"q{h}") for h in range(HPC)]
            k_sb = [qkv.tile([DH, T], BF16, name=f"k{h}") for h in range(HPC)]
            v_all = qkv.tile([KB, T // KB, C_LOC], BF16)

            def rope_panel(tp):
                """Anchor-relative rope on rows 0:2M of q/k panel tp."""
                sl = slice(tp * PANEL, (tp + 1) * PANEL)
                for u in [t for pair in zip(q_sb, k_sb) for t in pair]:
                    sw = work.tile([2 * M, PANEL], BF16, tag="ropesw", bufs=2)
                    nc.vector.tensor_copy(sw[0:M, :], u[M:2 * M, sl])
                    nc.vector.tensor_copy(sw[M:2 * M, :], u[0:M, sl])
                    nc.vector.tensor_mul(sw[:], sw[:], sinN[:, sl])
                    nc.vector.tensor_mul(u[0:2 * M, sl], u[0:2 * M, sl],
                                         cos64[:, sl])
                    nc.vector.tensor_add(u[0:2 * M, sl], u[0:2 * M, sl],
                                         sw[:])

            ones128 = const.tile([KB, KB], BF16)
            sinN = const.tile([2 * M, T], F32)
            cos64 = const.tile([2 * M, T], F32)
            mask128 = const.tile([KB, KB], BF16)

            # ---- qkv projection: direct bf16 loads, per-panel rope ----
            with tc.tile_pool(name="wload", bufs=1) as wload:
                # issue the big loads FIRST (one contiguous DMA per tensor,
                # x per quarter) so nothing gates the first matmul chain
                xbf = wload.tile([KB, n_panels, n_cb, PANEL], BF16)
                wqkb = wload.tile([KB, n_cb, 2 * C_LOC], BF16)
                wvb = wload.tile([KB, n_cb, C_LOC], BF16)
                # chunk along kb and push everything from ONE queue in strict
                # priority order (ring order = push order): wv+x0 first for
                # the v chains, wqk next, later panels and wo last
                for k4 in range(0, n_cb, 4):
                    ks = slice(k4, k4 + 4)
                    nc.sync.dma_start(out=wvb[:, ks, :], in_=wv_d[:, ks, :])
                    nc.sync.dma_start(out=xbf[:, 0, ks, :],
                                      in_=xT_d[0][:, ks, :])
                for k4 in range(0, n_cb, 4):
                    ks = slice(k4, k4 + 4)
                    nc.sync.dma_start(out=wqkb[:, ks, :],
                                      in_=wqk_d[:, ks, :])
                for qt in (1, 2, 3):
                    for k4 in range(0, n_cb, 4):
                        ks = slice(k4, k4 + 4)
                        nc.sync.dma_start(out=xbf[:, qt, ks, :],
                                          in_=xT_d[qt][:, ks, :])

                # ---- constants: trig tables, diag mask, ones ----
                nc.vector.memset(ones128[:], 1.0)
                # warm the PE HAM clock gate during the initial DMA wait so
                # the first real matmuls run at full rate
                with tc.tile_pool(name="warm", bufs=1, space="PSUM") as pwarm:
                    wt = pwarm.tile([KB, KB], F32, tag="w", bufs=1)
                    for _ in range(48):
                        nc.tensor.matmul(wt[:], ones128[:], ones128[:],
                                         start=True, stop=True)
                with tc.tile_pool(name="setup", bufs=1) as setup:
                    # fr64 = [-freqs; freqs] as per-partition scalars
                    fr64 = setup.tile([2 * M, 1], F32)
                    nc.sync.dma_start(out=fr64[0:M, :],
                                      in_=freqs_d.rearrange("m -> m ()"))
                    nc.sync.dma_start(out=fr64[M:2 * M, :],
                                      in_=freqs_d.rearrange("m -> m ()"))
                    nc.vector.tensor_scalar_mul(fr64[0:M, :], fr64[0:M, :],
                                                -1.0)

                    # delta replicated across 2M partitions via 0-stride DMA
                    delta_rep = setup.tile([2 * M, T], F32)
                    nc.sync.dma_start(
                        out=delta_rep[:],
                        in_=delta_d.rearrange("t -> () t")
                        .partition_broadcast(2 * M))

                    # ang = delta * (+-freqs) in place; sinN/cos via Sin
                    nc.vector.tensor_scalar_mul(delta_rep[:], delta_rep[:],
                                                fr64[:])
                    nc.scalar.activation(sinN[:], delta_rep[:],
                                         mybir.ActivationFunctionType.Sin)
                    pi2 = setup.tile([2 * M, 1], F32)
                    nc.vector.memset(pi2[:], math.pi / 2)
                    nc.scalar.activation(cos64[:], delta_rep[:],
                                         mybir.ActivationFunctionType.Sin,
                                         bias=pi2[:])

                    # diagonal-subblock causal mask: mask[ki, c] = (c >= ki)
                    mi = setup.tile([KB, KB], F32)
                    nc.gpsimd.iota(mi[:], pattern=[[1, KB]], base=0,
                                   channel_multiplier=-1,
                                   allow_small_or_imprecise_dtypes=True)
                    nc.vector.tensor_scalar(mask128[:], mi[:], 0.0, None,
                                            mybir.AluOpType.is_ge)

                with tc.tile_pool(name="psq", bufs=1, space="PSUM") as psq:
                    for tp in range(n_panels):
                        tps = tp * PANEL
                        # v blocks for the 128-rows inside this panel
                        for tbl in range(kb_per_panel):
                            tb = tp * kb_per_panel + tbl
                            pv = psq.tile([KB, C_LOC], F32, tag="v", bufs=3)
                            for kb in range(n_cb):
                                nc.tensor.matmul(
                                    pv[:],
                                    xbf[:, tp, kb, tbl * KB:(tbl + 1) * KB],
                                    wvb[:, kb, :],
                                    start=(kb == 0), stop=(kb == n_cb - 1))
                            nc.scalar.copy(v_all[:, tb, :], pv[:])
                        # q/k column blocks: cb<HPC -> q head cb; else k head
                        for cb in range(2 * HPC):
                            pqk = psq.tile([DH, PANEL], F32, tag="qk", bufs=3)
                            for kb in range(n_cb):
                                nc.tensor.matmul(
                                    pqk[:],
                                    wqkb[:, kb, cb * DH:(cb + 1) * DH],
                                    xbf[:, tp, kb, :],
                                    start=(kb == 0), stop=(kb == n_cb - 1))
                            dst = q_sb[cb] if cb < HPC else k_sb[cb - HPC]
                            nc.scalar.copy(dst[:, tps:tps + PANEL], pqk[:])
                        # rope panels 0/1 here; 2/3 are deferred into early
                        # attention (not needed until their query panel)
                        if tp < 2:
                            rope_panel(tp)

            # ---- attention (panel-outer, head-inner) + partial projection ----
            attn_cm = tc.tile_pool(name="attn", bufs=1)
            attn = attn_cm.__enter__()
            ysb_all = attn.tile([DH, HPC, T], BF16)
            wo_sb = attn.tile([KB, HPC, C], BF16)
            for cb in range(HPC):
                nc.gpsimd.dma_start(out=wo_sb[:, cb, :], in_=wo_d[:, cb, :])

            psa_cm = tc.tile_pool(name="psa", bufs=1, space="PSUM")
            psa = psa_cm.__enter__()

            def attention_unit(J, h):
                """Emit s/exp/mask/acc/av for panel J of head h; return the
                deferred finalize closure (rowsum-MM, normalize into ysb)."""
                qh, kh = q_sb[h], k_sb[h]
                qs = J * PANEL
                nkb = (J + 1) * kb_per_panel
                py = psa.tile([DH, PANEL], F32, tag="y", bufs=2)
                acc = work.tile([KB, PANEL], BF16, tag="acc", bufs=2)
                for b in range(nkb):
                    p = b - kb_per_panel * J
                    off = max(p, 0) * KB
                    ps = psa.tile([KB, PANEL], F32, tag="s", bufs=2)
                    nc.tensor.matmul(
                        ps[:, off:],
                        kh[:, b * KB:(b + 1) * KB],
                        qh[:, qs + off:qs + PANEL],
                        start=True, stop=True)
                    et = work.tile([KB, PANEL], BF16, tag="exp", bufs=4)
                    nc.scalar.activation(
                        et[:, off:], ps[:, off:],
                        mybir.ActivationFunctionType.Exp,
                        scale=inv_sqrt_d)
                    if p >= 0:
                        nc.vector.tensor_mul(et[:, off:off + KB],
                                             et[:, off:off + KB], mask128[:])
                    if b == 0:
                        nc.vector.tensor_copy(acc[:], et[:])
                    else:
                        nc.vector.tensor_add(acc[:, off:], acc[:, off:],
                                             et[:, off:])
                    nc.tensor.matmul(
                        py[:, off:],
                        v_all[:, b, h * DH:(h + 1) * DH],
                        et[:, off:],
                        start=(b == 0), stop=(b == nkb - 1))

                def finalize():
                    pr = psa.tile([KB, PANEL], F32, tag="r", bufs=1)
                    nc.tensor.matmul(pr[:], ones128[:], acc[:],
                                     start=True, stop=True)
                    rinv = work.tile([KB, PANEL], F32, tag="rinv", bufs=2)
                    nc.vector.reciprocal_approx_fast(rinv[:], pr[:])
                    nc.vector.tensor_mul(ysb_all[:, h, qs:qs + PANEL],
                                         py[:], rinv[:])
                return finalize

            def proj_panel(J):
                """partT[:, J panel] = sum_h wo[own h].T @ y[h, J panel]."""
                ts_ = J * PANEL
                for oc in range(n_oc):
                    po = psa.tile([KB, PANEL], F32, tag="po", bufs=3)
                    for h in range(HPC):
                        nc.tensor.matmul(
                            po[:],
                            wo_sb[:, h, oc * KB:(oc + 1) * KB],
                            ysb_all[:, h, ts_:ts_ + PANEL],
                            start=(h == 0), stop=(h == HPC - 1))
                    ost = work.tile([KB, PANEL], BF16, tag="ost", bufs=4)
                    if oc % 2 == 0:
                        nc.scalar.copy(ost[:], po[:])
                    else:
                        nc.vector.tensor_copy(ost[:], po[:])
                    nc.sync.dma_start(
                        out=partT_d[oc * KB:(oc + 1) * KB, ts_:ts_ + PANEL],
                        in_=ost[:])

            pending = None
            for J in range(n_panels):
                for h in range(HPC):
                    fin = attention_unit(J, h)
                    if pending is not None:
                        pending()
                    pending = fin
                    if h == 1 and J >= 1:
                        proj_panel(J - 1)
                if J < 2:
                    rope_panel(J + 2)
            pending()
            proj_panel(n_panels - 1)

            psa_cm.__exit__(None, None, None)
            attn_cm.__exit__(None, None, None)

    nc.compile()
    return nc


def _perm(a):
    """[C, cols] f32 -> [128, n_cb, cols] bf16 with c = kb*128 + p."""
    import ml_dtypes
    c, cols = a.shape
    return np.ascontiguousarray(
        a.reshape(c // KB, KB, cols).transpose(1, 0, 2)
    ).astype(ml_dtypes.bfloat16)


def make_in_maps(x, w_attn, w_proj, freqs, delta):
    """Host-side sharding: slice/transpose/convert full inputs per core."""
    x = np.asarray(x, dtype=np.float32)
    w_attn = np.asarray(w_attn, dtype=np.float32)
    w_proj = np.asarray(w_proj, dtype=np.float32)
    freqs = np.asarray(freqs, dtype=np.float32)
    delta = np.asarray(delta, dtype=np.float32)
    c_ = x.shape[2]
    in_maps = []
    for core in range(N_CORES):
        g, pos = divmod(core, CPG)
        heads = range(pos * HPC, (pos + 1) * HPC)
        # [qt, p, kb, 512] with c = kb*128 + p, t = qt*512 + tq
        xT = np.ascontiguousarray(
            _perm(np.ascontiguousarray(x[g].T))
            .reshape(KB, C // KB, T // PANEL, PANEL).transpose(2, 0, 1, 3))
        wqk = _perm(np.concatenate(
            [w_attn[:, h * DH:(h + 1) * DH] for h in heads]
            + [w_attn[:, c_ + h * DH:c_ + (h + 1) * DH] for h in heads],
            axis=1))
        wv = _perm(np.ascontiguousarray(
            w_attn[:, 2 * c_ + pos * C_LOC:2 * c_ + (pos + 1) * C_LOC]))
        wo = _perm(np.ascontiguousarray(
            w_proj[pos * C_LOC:(pos + 1) * C_LOC, :]))
        in_maps.append({
            "xT": xT, "wqk": wqk, "wv": wv, "wo": wo,
            "freqs": freqs, "delta": delta,
        })
    return in_maps


def assemble_output(results):
    outs = []
    for g in range(GROUPS):
        acc = results[g * CPG]["partT"].astype(np.float32)
        for pos in range(1, CPG):
            acc = acc + results[g * CPG + pos]["partT"].astype(np.float32)
        outs.append(acc.T)
    return np.stack(outs, axis=0).astype(np.float32)


_NC_CACHE = {}


def _get_program():
    if "nc" not in _NC_CACHE:
        _NC_CACHE["nc"] = build_program()
    return _NC_CACHE["nc"]


def kernel(x, w_attn, w_proj, freqs, delta):
    nc = _get_program()
    in_maps = make_in_maps(x, w_attn, w_proj, freqs, delta)
    res = run_bass_kernel_spmd(nc, in_maps, list(range(N_CORES)))
    return assemble_output(res.results)


# revision 27
# speedup vs baseline: 1.0196x; 1.0196x over previous
"""Causal self-attention with anchor-relative rope (ferope), 8-core TRN2 Bass kernel.

Full-scale problem: B=2, T=2048, C=2048, H=16, D=128, M=32.

Sharding (tensor-parallel heads + data-parallel batch), collective-free:
  - 8 cores = 2 batch groups x 4 cores. Core (b, g) handles batch b, heads 4g..4g+3.
  - All matrix inputs are pre-converted to bf16 and pre-permuted on the host so
    each contraction block [128, .] DMAs contiguously into SBUF (no staging).
  - qkv projection: per-core column shard of w_attn; q/k produced in [d, t]
    layout, v in [t, d]; rope applied per 512-panel right after projection.
  - attention runs query-panel-outer / head-inner with transposed scores
    s_T[ki, qi]; diagonal blocks narrowed to the exact causal triangle;
    softmax denominator accumulated in bf16 on the vector engine + one
    ones-matmul per panel; finalization deferred one unit to avoid stalls.
  - output projection needs no AllGather: each core computes the full-width
    partial out^T = wo_own^T @ y_own (same flops as a sharded projection,
    contraction over its own 512 channels) and the host sums the 4 partials
    per batch group while unsharding.
"""

import math

import numpy as np

import concourse.bass as bass
import concourse.mybir as mybir
import concourse.tile as tile
from concourse import bacc
from concourse.bass_utils import run_bass_kernel_spmd

F32 = mybir.dt.float32
BF16 = mybir.dt.bfloat16

# full-scale dims (hardcoded per harness contract)
B, T, C, H, DH, M = 2, 2048, 2048, 16, 128, 32
N_CORES = 8
GROUPS = 2                     # batch groups
CPG = N_CORES // GROUPS        # cores per group = 4
HPC = H // CPG                 # heads per core = 4
C_LOC = HPC * DH               # 512: per-core head channels
PANEL = 512                    # qi panel width (one psum bank)
KB = 128                       # ki block (partition dim)


def build_program():
    n_cb = C // KB              # 16 contraction blocks for qkv
    n_oc = C // KB              # 16 output-column blocks for proj
    n_panels = T // PANEL       # 4
    kb_per_panel = PANEL // KB  # 4
    inv_sqrt_d = 1.0 / math.sqrt(DH)

    nc = bacc.Bacc("TRN2", target_bir_lowering=False, debug=False,
                   num_devices=N_CORES)

    # pre-permuted bf16 inputs, contiguous per partition row so each tensor
    # loads with one big-descriptor DMA. x is quarter-major: [qt, p, kb, 512]
    xT_d = nc.dram_tensor("xT", [n_panels, KB, n_cb, PANEL], BF16,
                          kind="ExternalInput").ap()
    wqk_d = nc.dram_tensor("wqk", [KB, n_cb, 2 * C_LOC], BF16,
                           kind="ExternalInput").ap()
    wv_d = nc.dram_tensor("wv", [KB, n_cb, C_LOC], BF16,
                          kind="ExternalInput").ap()
    # proj weight rows for this core's channels: [p, own-cblk, out-cols]
    wo_d = nc.dram_tensor("wo", [KB, HPC, C], BF16, kind="ExternalInput").ap()
    freqs_d = nc.dram_tensor("freqs", [M], F32, kind="ExternalInput").ap()
    delta_d = nc.dram_tensor("delta", [T], F32, kind="ExternalInput").ap()
    # full-width transposed partial projection; host sums over the group
    partT_d = nc.dram_tensor("partT", [C, T], BF16, kind="ExternalOutput").ap()

    with tile.TileContext(nc) as tc:
        with (
            tc.tile_pool(name="const", bufs=1) as const,
            tc.tile_pool(name="qkv", bufs=1) as qkv,
            tc.tile_pool(name="work", bufs=1) as work,
        ):
            # persistent attention operands
            q_sb = [qkv.tile([DH, T], BF16, name=f"q{h}") for h in range(HPC)]
            k_sb = [qkv.tile([DH, T], BF16, name=f"k{h}") for h in range(HPC)]
            v_all = qkv.tile([KB, T // KB, C_LOC], BF16)

            def rope_panel(tp):
                """Anchor-relative rope on rows 0:2M of q/k panel tp."""
                sl = slice(tp * PANEL, (tp + 1) * PANEL)
                for u in [t for pair in zip(q_sb, k_sb) for t in pair]:
                    sw = work.tile([2 * M, PANEL], BF16, tag="ropesw", bufs=2)
                    nc.vector.tensor_copy(sw[0:M, :], u[M:2 * M, sl])
                    nc.vector.tensor_copy(sw[M:2 * M, :], u[0:M, sl])
                    nc.vector.tensor_mul(sw[:], sw[:], sinN[:, sl])
                    nc.vector.tensor_mul(u[0:2 * M, sl], u[0:2 * M, sl],
                                         cos64[:, sl])
                    nc.vector.tensor_add(u[0:2 * M, sl], u[0:2 * M, sl],
                                         sw[:])

            ones128 = const.tile([KB, KB], BF16)
            sinN = const.tile([2 * M, T], F32)
            cos64 = const.tile([2 * M, T], F32)
            mask128 = const.tile([KB, KB], BF16)

            # ---- qkv projection: direct bf16 loads, per-panel rope ----
            with tc.tile_pool(name="wload", bufs=1) as wload:
                # issue the big loads FIRST (one contiguous DMA per tensor,
                # x per quarter) so nothing gates the first matmul chain
                xbf = wload.tile([KB, n_panels, n_cb, PANEL], BF16)
                wqkb = wload.tile([KB, n_cb, 2 * C_LOC], BF16)
                wvb = wload.tile([KB, n_cb, C_LOC], BF16)
                # chunk along kb and push everything from ONE queue in strict
                # priority order (ring order = push order): wv+x0 first for
                # the v chains, wqk next, later panels and wo last
                for k4 in range(0, n_cb, 4):
                    ks = slice(k4, k4 + 4)
                    nc.sync.dma_start(out=wvb[:, ks, :], in_=wv_d[:, ks, :])
                    nc.sync.dma_start(out=xbf[:, 0, ks, :],
                                      in_=xT_d[0][:, ks, :])
                    nc.sync.dma_start(out=wqkb[:, ks, :],
                                      in_=wqk_d[:, ks, :])
                for qt in (1, 2, 3):
                    for k4 in range(0, n_cb, 4):
                        ks = slice(k4, k4 + 4)
                        nc.sync.dma_start(out=xbf[:, qt, ks, :],
                                          in_=xT_d[qt][:, ks, :])

                # ---- constants: trig tables, diag mask, ones ----
                nc.vector.memset(ones128[:], 1.0)
                # warm the PE HAM clock gate during the initial DMA wait so
                # the first real matmuls run at full rate
                with tc.tile_pool(name="warm", bufs=1, space="PSUM") as pwarm:
                    wt = pwarm.tile([KB, KB], F32, tag="w", bufs=1)
                    for _ in range(48):
                        nc.tensor.matmul(wt[:], ones128[:], ones128[:],
                                         start=True, stop=True)
                with tc.tile_pool(name="setup", bufs=1) as setup:
                    # fr64 = [-freqs; freqs] as per-partition scalars
                    fr64 = setup.tile([2 * M, 1], F32)
                    nc.sync.dma_start(out=fr64[0:M, :],
                                      in_=freqs_d.rearrange("m -> m ()"))
                    nc.sync.dma_start(out=fr64[M:2 * M, :],
                                      in_=freqs_d.rearrange("m -> m ()"))
                    nc.vector.tensor_scalar_mul(fr64[0:M, :], fr64[0:M, :],
                                                -1.0)

                    # delta replicated across 2M partitions via 0-stride DMA
                    delta_rep = setup.tile([2 * M, T], F32)
                    nc.sync.dma_start(
                        out=delta_rep[:],
                        in_=delta_d.rearrange("t -> () t")
                        .partition_broadcast(2 * M))

                    # ang = delta * (+-freqs) in place; sinN/cos via Sin
                    nc.vector.tensor_scalar_mul(delta_rep[:], delta_rep[:],
                                                fr64[:])
                    nc.scalar.activation(sinN[:], delta_rep[:],
                                         mybir.ActivationFunctionType.Sin)
                    pi2 = setup.tile([2 * M, 1], F32)
                    nc.vector.memset(pi2[:], math.pi / 2)
                    nc.scalar.activation(cos64[:], delta_rep[:],
                                         mybir.ActivationFunctionType.Sin,
                                         bias=pi2[:])

                    # diagonal-subblock causal mask: mask[ki, c] = (c >= ki)
                    mi = setup.tile([KB, KB], F32)
                    nc.gpsimd.iota(mi[:], pattern=[[1, KB]], base=0,
                                   channel_multiplier=-1,
                                   allow_small_or_imprecise_dtypes=True)
                    nc.vector.tensor_scalar(mask128[:], mi[:], 0.0, None,
                                            mybir.AluOpType.is_ge)

                with tc.tile_pool(name="psq", bufs=1, space="PSUM") as psq:
                    for tp in range(n_panels):
                        tps = tp * PANEL
                        # v blocks for the 128-rows inside this panel
                        for tbl in range(kb_per_panel):
                            tb = tp * kb_per_panel + tbl
                            pv = psq.tile([KB, C_LOC], F32, tag="v", bufs=3)
                            for kb in range(n_cb):
                                nc.tensor.matmul(
                                    pv[:],
                                    xbf[:, tp, kb, tbl * KB:(tbl + 1) * KB],
                                    wvb[:, kb, :],
                                    start=(kb == 0), stop=(kb == n_cb - 1))
                            nc.scalar.copy(v_all[:, tb, :], pv[:])
                        # q/k column blocks: cb<HPC -> q head cb; else k head
                        for cb in range(2 * HPC):
                            pqk = psq.tile([DH, PANEL], F32, tag="qk", bufs=3)
                            for kb in range(n_cb):
                                nc.tensor.matmul(
                                    pqk[:],
                                    wqkb[:, kb, cb * DH:(cb + 1) * DH],
                                    xbf[:, tp, kb, :],
                                    start=(kb == 0), stop=(kb == n_cb - 1))
                            dst = q_sb[cb] if cb < HPC else k_sb[cb - HPC]
                            nc.scalar.copy(dst[:, tps:tps + PANEL], pqk[:])
                        # rope panels 0/1 here; 2/3 are deferred into early
                        # attention (not needed until their query panel)
                        if tp < 2:
                            rope_panel(tp)

            # ---- attention (panel-outer, head-inner) + partial projection ----
            attn_cm = tc.tile_pool(name="attn", bufs=1)
            attn = attn_cm.__enter__()
            ysb_all = attn.tile([DH, HPC, T], BF16)
            wo_sb = attn.tile([KB, HPC, C], BF16)
            for cb in range(HPC):
                nc.gpsimd.dma_start(out=wo_sb[:, cb, :], in_=wo_d[:, cb, :])

            psa_cm = tc.tile_pool(name="psa", bufs=1, space="PSUM")
            psa = psa_cm.__enter__()

            def attention_unit(J, h):
                """Emit s/exp/mask/acc/av for panel J of head h; return the
                deferred finalize closure (rowsum-MM, normalize into ysb)."""
                qh, kh = q_sb[h], k_sb[h]
                qs = J * PANEL
                nkb = (J + 1) * kb_per_panel
                py = psa.tile([DH, PANEL], F32, tag="y", bufs=2)
                acc = work.tile([KB, PANEL], BF16, tag="acc", bufs=2)
                for b in range(nkb):
                    p = b - kb_per_panel * J
                    off = max(p, 0) * KB
                    ps = psa.tile([KB, PANEL], F32, tag="s", bufs=2)
                    nc.tensor.matmul(
                        ps[:, off:],
                        kh[:, b * KB:(b + 1) * KB],
                        qh[:, qs + off:qs + PANEL],
                        start=True, stop=True)
                    et = work.tile([KB, PANEL], BF16, tag="exp", bufs=4)
                    nc.scalar.activation(
                        et[:, off:], ps[:, off:],
                        mybir.ActivationFunctionType.Exp,
                        scale=inv_sqrt_d)
                    if p >= 0:
                        nc.vector.tensor_mul(et[:, off:off + KB],
                                             et[:, off:off + KB], mask128[:])
                    if b == 0:
                        nc.vector.tensor_copy(acc[:], et[:])
                    else:
                        nc.vector.tensor_add(acc[:, off:], acc[:, off:],
                                             et[:, off:])
                    nc.tensor.matmul(
                        py[:, off:],
                        v_all[:, b, h * DH:(h + 1) * DH],
                        et[:, off:],
                        start=(b == 0), stop=(b == nkb - 1))

                def finalize():
                    pr = psa.tile([KB, PANEL], F32, tag="r", bufs=1)
                    nc.tensor.matmul(pr[:], ones128[:], acc[:],
                                     start=True, stop=True)
                    rinv = work.tile([KB, PANEL], F32, tag="rinv", bufs=2)
                    nc.vector.reciprocal_approx_fast(rinv[:], pr[:])
                    nc.vector.tensor_mul(ysb_all[:, h, qs:qs + PANEL],
                                         py[:], rinv[:])
                return finalize

            def proj_panel(J):
                """partT[:, J panel] = sum_h wo[own h].T @ y[h, J panel]."""
                ts_ = J * PANEL
                for oc in range(n_oc):
                    po = psa.tile([KB, PANEL], F32, tag="po", bufs=3)
                    for h in range(HPC):
                        nc.tensor.matmul(
                            po[:],
                            wo_sb[:, h, oc * KB:(oc + 1) * KB],
                            ysb_all[:, h, ts_:ts_ + PANEL],
                            start=(h == 0), stop=(h == HPC - 1))
                    ost = work.tile([KB, PANEL], BF16, tag="ost", bufs=4)
                    if oc % 2 == 0:
                        nc.scalar.copy(ost[:], po[:])
                    else:
                        nc.vector.tensor_copy(ost[:], po[:])
                    nc.sync.dma_start(
                        out=partT_d[oc * KB:(oc + 1) * KB, ts_:ts_ + PANEL],
                        in_=ost[:])

            pending = None
            for J in range(n_panels):
                for h in range(HPC):
                    fin = attention_unit(J, h)
                    if pending is not None:
                        pending()
                    pending = fin
                    if h == 1 and J >= 1:
                        proj_panel(J - 1)
                if J < 2:
                    rope_panel(J + 2)
            pending()
            proj_panel(n_panels - 1)

            psa_cm.__exit__(None, None, None)
            attn_cm.__exit__(None, None, None)

    nc.compile()
    return nc


def _perm(a):
    """[C, cols] f32 -> [128, n_cb, cols] bf16 with c = kb*128 + p."""
    import ml_dtypes
    c, cols = a.shape
    return np.ascontiguousarray(
        a.reshape(c // KB, KB, cols).transpose(1, 0, 2)
    ).astype(ml_dtypes.bfloat16)


def make_in_maps(x, w_attn, w_proj, freqs, delta):
    """Host-side sharding: slice/transpose/convert full inputs per core."""
    x = np.asarray(x, dtype=np.float32)
    w_attn = np.asarray(w_attn, dtype=np.float32)
    w_proj = np.asarray(w_proj, dtype=np.float32)
    freqs = np.asarray(freqs, dtype=np.float32)
    delta = np.asarray(delta, dtype=np.float32)
    c_ = x.shape[2]
    in_maps = []
    for core in range(N_CORES):
        g, pos = divmod(core, CPG)
        heads = range(pos * HPC, (pos + 1) * HPC)
        # [qt, p, kb, 512] with c = kb*128 + p, t = qt*512 + tq
        xT = np.ascontiguousarray(
            _perm(np.ascontiguousarray(x[g].T))
            .reshape(KB, C // KB, T // PANEL, PANEL).transpose(2, 0, 1, 3))
        wqk = _perm(np.concatenate(
            [w_attn[:, h * DH:(h + 1) * DH] for h in heads]
            + [w_attn[:, c_ + h * DH:c_ + (h + 1) * DH] for h in heads],
            axis=1))
        wv = _perm(np.ascontiguousarray(
            w_attn[:, 2 * c_ + pos * C_LOC:2 * c_ + (pos + 1) * C_LOC]))
        wo = _perm(np.ascontiguousarray(
            w_proj[pos * C_LOC:(pos + 1) * C_LOC, :]))
        in_maps.append({
            "xT": xT, "wqk": wqk, "wv": wv, "wo": wo,
            "freqs": freqs, "delta": delta,
        })
    return in_maps


def assemble_output(results):
    outs = []
    for g in range(GROUPS):
        acc = results[g * CPG]["partT"].astype(np.float32)
        for pos in range(1, CPG):
            acc = acc + results[g * CPG + pos]["partT"].astype(np.float32)
        outs.append(acc.T)
    return np.stack(outs, axis=0).astype(np.float32)


_NC_CACHE = {}


def _get_program():
    if "nc" not in _NC_CACHE:
        _NC_CACHE["nc"] = build_program()
    return _NC_CACHE["nc"]


def kernel(x, w_attn, w_proj, freqs, delta):
    nc = _get_program()
    in_maps = make_in_maps(x, w_attn, w_proj, freqs, delta)
    res = run_bass_kernel_spmd(nc, in_maps, list(range(N_CORES)))
    return assemble_output(res.results)


# revision 28
# speedup vs baseline: 1.0372x; 1.0173x over previous
"""Causal self-attention with anchor-relative rope (ferope), 8-core TRN2 Bass kernel.

Full-scale problem: B=2, T=2048, C=2048, H=16, D=128, M=32.

Sharding (tensor-parallel heads + data-parallel batch), collective-free:
  - 8 cores = 2 batch groups x 4 cores. Core (b, g) handles batch b, heads 4g..4g+3.
  - All matrix inputs are pre-converted to bf16 and pre-permuted on the host so
    each contraction block [128, .] DMAs contiguously into SBUF (no staging).
  - qkv projection: per-core column shard of w_attn; q/k produced in [d, t]
    layout, v in [t, d]; rope applied per 512-panel right after projection.
  - attention runs query-panel-outer / head-inner with transposed scores
    s_T[ki, qi]; diagonal blocks narrowed to the exact causal triangle;
    softmax denominator accumulated in bf16 on the vector engine + one
    ones-matmul per panel; finalization deferred one unit to avoid stalls.
  - output projection needs no AllGather: each core computes the full-width
    partial out^T = wo_own^T @ y_own (same flops as a sharded projection,
    contraction over its own 512 channels) and the host sums the 4 partials
    per batch group while unsharding.
"""

import math

import numpy as np

import concourse.bass as bass
import concourse.mybir as mybir
import concourse.tile as tile
from concourse import bacc
from concourse.bass_utils import run_bass_kernel_spmd

F32 = mybir.dt.float32
BF16 = mybir.dt.bfloat16

# full-scale dims (hardcoded per harness contract)
B, T, C, H, DH, M = 2, 2048, 2048, 16, 128, 32
N_CORES = 8
GROUPS = 2                     # batch groups
CPG = N_CORES // GROUPS        # cores per group = 4
HPC = H // CPG                 # heads per core = 4
C_LOC = HPC * DH               # 512: per-core head channels
PANEL = 512                    # qi panel width (one psum bank)
KB = 128                       # ki block (partition dim)


def build_program():
    n_cb = C // KB              # 16 contraction blocks for qkv
    n_oc = C // KB              # 16 output-column blocks for proj
    n_panels = T // PANEL       # 4
    kb_per_panel = PANEL // KB  # 4
    inv_sqrt_d = 1.0 / math.sqrt(DH)

    nc = bacc.Bacc("TRN2", target_bir_lowering=False, debug=False,
                   num_devices=N_CORES)

    # pre-permuted bf16 inputs, contiguous per partition row so each tensor
    # loads with one big-descriptor DMA. x is quarter-major: [qt, p, kb, 512]
    xT_d = nc.dram_tensor("xT", [n_panels, KB, n_cb, PANEL], BF16,
                          kind="ExternalInput").ap()
    wqk_d = nc.dram_tensor("wqk", [KB, n_cb, 2 * C_LOC], BF16,
                           kind="ExternalInput").ap()
    wv_d = nc.dram_tensor("wv", [KB, n_cb, C_LOC], BF16,
                          kind="ExternalInput").ap()
    # proj weight rows for this core's channels: [p, own-cblk, out-cols]
    wo_d = nc.dram_tensor("wo", [KB, HPC, C], BF16, kind="ExternalInput").ap()
    freqs_d = nc.dram_tensor("freqs", [M], F32, kind="ExternalInput").ap()
    delta_d = nc.dram_tensor("delta", [T], F32, kind="ExternalInput").ap()
    # full-width transposed partial projection; host sums over the group
    partT_d = nc.dram_tensor("partT", [C, T], BF16, kind="ExternalOutput").ap()

    with tile.TileContext(nc) as tc:
        with (
            tc.tile_pool(name="const", bufs=1) as const,
            tc.tile_pool(name="qkv", bufs=1) as qkv,
            tc.tile_pool(name="work", bufs=1) as work,
        ):
            # persistent attention operands
            q_sb = [qkv.tile([DH, T], BF16, name=f"q{h}") for h in range(HPC)]
            k_sb = [qkv.tile([DH, T], BF16, name=f"k{h}") for h in range(HPC)]
            v_all = qkv.tile([KB, T // KB, C_LOC], BF16)

            def rope_panel(tp):
                """Anchor-relative rope on rows 0:2M of q/k panel tp."""
                sl = slice(tp * PANEL, (tp + 1) * PANEL)
                for u in [t for pair in zip(q_sb, k_sb) for t in pair]:
                    sw = work.tile([2 * M, PANEL], BF16, tag="ropesw", bufs=2)
                    nc.vector.tensor_copy(sw[0:M, :], u[M:2 * M, sl])
                    nc.vector.tensor_copy(sw[M:2 * M, :], u[0:M, sl])
                    nc.vector.tensor_mul(sw[:], sw[:], sinN[:, sl])
                    nc.vector.tensor_mul(u[0:2 * M, sl], u[0:2 * M, sl],
                                         cos64[:, sl])
                    nc.vector.tensor_add(u[0:2 * M, sl], u[0:2 * M, sl],
                                         sw[:])

            ones128 = const.tile([KB, KB], BF16)
            sinN = const.tile([2 * M, T], F32)
            cos64 = const.tile([2 * M, T], F32)
            mask128 = const.tile([KB, KB], BF16)

            # ---- qkv projection: direct bf16 loads, per-panel rope ----
            with tc.tile_pool(name="wload", bufs=1) as wload:
                # issue the big loads FIRST (one contiguous DMA per tensor,
                # x per quarter) so nothing gates the first matmul chain
                xbf = wload.tile([KB, n_panels, n_cb, PANEL], BF16)
                wqkb = wload.tile([KB, n_cb, 2 * C_LOC], BF16)
                wvb = wload.tile([KB, n_cb, C_LOC], BF16)
                # chunk along kb and push everything from ONE queue in strict
                # priority order (ring order = push order): wv+x0 first for
                # the v chains, wqk next, later panels and wo last
                for k4 in range(0, n_cb, 4):
                    ks = slice(k4, k4 + 4)
                    nc.sync.dma_start(out=wvb[:, ks, :], in_=wv_d[:, ks, :])
                    nc.sync.dma_start(out=xbf[:, 0, ks, :],
                                      in_=xT_d[0][:, ks, :])
                    nc.sync.dma_start(out=wqkb[:, ks, :],
                                      in_=wqk_d[:, ks, :])
                for qt in (1, 2, 3):
                    for k4 in range(0, n_cb, 4):
                        ks = slice(k4, k4 + 4)
                        nc.sync.dma_start(out=xbf[:, qt, ks, :],
                                          in_=xT_d[qt][:, ks, :])

                # ---- constants: trig tables, diag mask, ones ----
                nc.vector.memset(ones128[:], 1.0)
                # warm the PE HAM clock gate during the initial DMA wait so
                # the first real matmuls run at full rate
                with tc.tile_pool(name="warm", bufs=1, space="PSUM") as pwarm:
                    wt = pwarm.tile([KB, KB], F32, tag="w", bufs=1)
                    for _ in range(48):
                        nc.tensor.matmul(wt[:], ones128[:], ones128[:],
                                         start=True, stop=True)
                with tc.tile_pool(name="setup", bufs=1) as setup:
                    # fr64 = [-freqs; freqs] as per-partition scalars
                    fr64 = setup.tile([2 * M, 1], F32)
                    nc.sync.dma_start(out=fr64[0:M, :],
                                      in_=freqs_d.rearrange("m -> m ()"))
                    nc.sync.dma_start(out=fr64[M:2 * M, :],
                                      in_=freqs_d.rearrange("m -> m ()"))
                    nc.vector.tensor_scalar_mul(fr64[0:M, :], fr64[0:M, :],
                                                -1.0)

                    # delta replicated across 2M partitions via 0-stride DMA
                    delta_rep = setup.tile([2 * M, T], F32)
                    nc.sync.dma_start(
                        out=delta_rep[:],
                        in_=delta_d.rearrange("t -> () t")
                        .partition_broadcast(2 * M))

                    # ang = delta * (+-freqs) in place; sinN/cos via Sin
                    nc.vector.tensor_scalar_mul(delta_rep[:], delta_rep[:],
                                                fr64[:])
                    nc.scalar.activation(sinN[:], delta_rep[:],
                                         mybir.ActivationFunctionType.Sin)
                    pi2 = setup.tile([2 * M, 1], F32)
                    nc.vector.memset(pi2[:], math.pi / 2)
                    nc.scalar.activation(cos64[:], delta_rep[:],
                                         mybir.ActivationFunctionType.Sin,
                                         bias=pi2[:])

                    # diagonal-subblock causal mask: mask[ki, c] = (c >= ki)
                    mi = setup.tile([KB, KB], F32)
                    nc.gpsimd.iota(mi[:], pattern=[[1, KB]], base=0,
                                   channel_multiplier=-1,
                                   allow_small_or_imprecise_dtypes=True)
                    nc.vector.tensor_scalar(mask128[:], mi[:], 0.0, None,
                                            mybir.AluOpType.is_ge)

                with tc.tile_pool(name="psq", bufs=1, space="PSUM") as psq:

                    def v_chains(tp, last):
                        for tbl in range(kb_per_panel):
                            tb = tp * kb_per_panel + tbl
                            pv = psq.tile([KB, C_LOC], F32, tag="v", bufs=3)
                            for kb in range(n_cb):
                                nc.tensor.matmul(
                                    pv[:],
                                    xbf[:, tp, kb, tbl * KB:(tbl + 1) * KB],
                                    wvb[:, kb, :],
                                    start=(kb == 0), stop=(kb == n_cb - 1))
                            # last panel: copy on DVE so the scalar engine is
                            # drained when attention's first exp arrives
                            if last:
                                nc.vector.tensor_copy(v_all[:, tb, :], pv[:])
                            else:
                                nc.scalar.copy(v_all[:, tb, :], pv[:])

                    def qk_chains(tp):
                        tps = tp * PANEL
                        # cb<HPC -> q head cb; else k head cb-HPC
                        for cb in range(2 * HPC):
                            pqk = psq.tile([DH, PANEL], F32, tag="qk", bufs=3)
                            for kb in range(n_cb):
                                nc.tensor.matmul(
                                    pqk[:],
                                    wqkb[:, kb, cb * DH:(cb + 1) * DH],
                                    xbf[:, tp, kb, :],
                                    start=(kb == 0), stop=(kb == n_cb - 1))
                            dst = q_sb[cb] if cb < HPC else k_sb[cb - HPC]
                            nc.scalar.copy(dst[:, tps:tps + PANEL], pqk[:])

                    for tp in range(n_panels):
                        if tp == n_panels - 1:
                            # last panel: qk first so its scalar copies drain
                            # behind the v matmuls before attention starts
                            qk_chains(tp)
                            v_chains(tp, last=True)
                        else:
                            v_chains(tp, last=False)
                            qk_chains(tp)
                        # rope panels 0/1 here; 2/3 are deferred into early
                        # attention (not needed until their query panel)
                        if tp < 2:
                            rope_panel(tp)

            # ---- attention (panel-outer, head-inner) + partial projection ----
            attn_cm = tc.tile_pool(name="attn", bufs=1)
            attn = attn_cm.__enter__()
            ysb_all = attn.tile([DH, HPC, T], BF16)
            wo_sb = attn.tile([KB, HPC, C], BF16)
            for cb in range(HPC):
                nc.gpsimd.dma_start(out=wo_sb[:, cb, :], in_=wo_d[:, cb, :])

            psa_cm = tc.tile_pool(name="psa", bufs=1, space="PSUM")
            psa = psa_cm.__enter__()

            def attention_unit(J, h):
                """Emit s/exp/mask/acc/av for panel J of head h; return the
                deferred finalize closure (rowsum-MM, normalize into ysb)."""
                qh, kh = q_sb[h], k_sb[h]
                qs = J * PANEL
                nkb = (J + 1) * kb_per_panel
                py = psa.tile([DH, PANEL], F32, tag="y", bufs=2)
                acc = work.tile([KB, PANEL], BF16, tag="acc", bufs=2)
                for b in range(nkb):
                    p = b - kb_per_panel * J
                    off = max(p, 0) * KB
                    ps = psa.tile([KB, PANEL], F32, tag="s", bufs=2)
                    nc.tensor.matmul(
                        ps[:, off:],
                        kh[:, b * KB:(b + 1) * KB],
                        qh[:, qs + off:qs + PANEL],
                        start=True, stop=True)
                    et = work.tile([KB, PANEL], BF16, tag="exp", bufs=4)
                    nc.scalar.activation(
                        et[:, off:], ps[:, off:],
                        mybir.ActivationFunctionType.Exp,
                        scale=inv_sqrt_d)
                    if p >= 0:
                        nc.vector.tensor_mul(et[:, off:off + KB],
                                             et[:, off:off + KB], mask128[:])
                    if b == 0:
                        nc.vector.tensor_copy(acc[:], et[:])
                    else:
                        nc.vector.tensor_add(acc[:, off:], acc[:, off:],
                                             et[:, off:])
                    nc.tensor.matmul(
                        py[:, off:],
                        v_all[:, b, h * DH:(h + 1) * DH],
                        et[:, off:],
                        start=(b == 0), stop=(b == nkb - 1))

                def finalize():
                    pr = psa.tile([KB, PANEL], F32, tag="r", bufs=1)
                    nc.tensor.matmul(pr[:], ones128[:], acc[:],
                                     start=True, stop=True)
                    rinv = work.tile([KB, PANEL], F32, tag="rinv", bufs=2)
                    nc.vector.reciprocal_approx_fast(rinv[:], pr[:])
                    nc.vector.tensor_mul(ysb_all[:, h, qs:qs + PANEL],
                                         py[:], rinv[:])
                return finalize

            def proj_panel(J):
                """partT[:, J panel] = sum_h wo[own h].T @ y[h, J panel]."""
                ts_ = J * PANEL
                for oc in range(n_oc):
                    po = psa.tile([KB, PANEL], F32, tag="po", bufs=3)
                    for h in range(HPC):
                        nc.tensor.matmul(
                            po[:],
                            wo_sb[:, h, oc * KB:(oc + 1) * KB],
                            ysb_all[:, h, ts_:ts_ + PANEL],
                            start=(h == 0), stop=(h == HPC - 1))
                    ost = work.tile([KB, PANEL], BF16, tag="ost", bufs=4)
                    if oc % 2 == 0:
                        nc.scalar.copy(ost[:], po[:])
                    else:
                        nc.vector.tensor_copy(ost[:], po[:])
                    nc.sync.dma_start(
                        out=partT_d[oc * KB:(oc + 1) * KB, ts_:ts_ + PANEL],
                        in_=ost[:])

            pending = None
            for J in range(n_panels):
                for h in range(HPC):
                    fin = attention_unit(J, h)
                    if pending is not None:
                        pending()
                    pending = fin
                    if h == 1 and J >= 1:
                        proj_panel(J - 1)
                if J < 2:
                    rope_panel(J + 2)
            pending()
            proj_panel(n_panels - 1)

            psa_cm.__exit__(None, None, None)
            attn_cm.__exit__(None, None, None)

    nc.compile()
    return nc


def _perm(a):
    """[C, cols] f32 -> [128, n_cb, cols] bf16 with c = kb*128 + p."""
    import ml_dtypes
    c, cols = a.shape
    return np.ascontiguousarray(
        a.reshape(c // KB, KB, cols).transpose(1, 0, 2)
    ).astype(ml_dtypes.bfloat16)


def make_in_maps(x, w_attn, w_proj, freqs, delta):
    """Host-side sharding: slice/transpose/convert full inputs per core."""
    x = np.asarray(x, dtype=np.float32)
    w_attn = np.asarray(w_attn, dtype=np.float32)
    w_proj = np.asarray(w_proj, dtype=np.float32)
    freqs = np.asarray(freqs, dtype=np.float32)
    delta = np.asarray(delta, dtype=np.float32)
    c_ = x.shape[2]
    in_maps = []
    for core in range(N_CORES):
        g, pos = divmod(core, CPG)
        heads = range(pos * HPC, (pos + 1) * HPC)
        # [qt, p, kb, 512] with c = kb*128 + p, t = qt*512 + tq
        xT = np.ascontiguousarray(
            _perm(np.ascontiguousarray(x[g].T))
            .reshape(KB, C // KB, T // PANEL, PANEL).transpose(2, 0, 1, 3))
        wqk = _perm(np.concatenate(
            [w_attn[:, h * DH:(h + 1) * DH] for h in heads]
            + [w_attn[:, c_ + h * DH:c_ + (h + 1) * DH] for h in heads],
            axis=1))
        wv = _perm(np.ascontiguousarray(
            w_attn[:, 2 * c_ + pos * C_LOC:2 * c_ + (pos + 1) * C_LOC]))
        wo = _perm(np.ascontiguousarray(
            w_proj[pos * C_LOC:(pos + 1) * C_LOC, :]))
        in_maps.append({
            "xT": xT, "wqk": wqk, "wv": wv, "wo": wo,
            "freqs": freqs, "delta": delta,
        })
    return in_maps


def assemble_output(results):
    outs = []
    for g in range(GROUPS):
        acc = results[g * CPG]["partT"].astype(np.float32)
        for pos in range(1, CPG):
            acc = acc + results[g * CPG + pos]["partT"].astype(np.float32)
        outs.append(acc.T)
    return np.stack(outs, axis=0).astype(np.float32)


_NC_CACHE = {}


def _get_program():
    if "nc" not in _NC_CACHE:
        _NC_CACHE["nc"] = build_program()
    return _NC_CACHE["nc"]


def kernel(x, w_attn, w_proj, freqs, delta):
    nc = _get_program()
    in_maps = make_in_maps(x, w_attn, w_proj, freqs, delta)
    res = run_bass_kernel_spmd(nc, in_maps, list(range(N_CORES)))
    return assemble_output(res.results)
